# revision 32
# baseline (speedup 1.0000x reference)
"""M2M-GNN (nn_M2MGNNPro) Trainium2 kernel, 8-core SPMD, bf16. v3.

Strategy (edge-parallel, destination-sharded, slot-permuted windows):
- Core k owns dest nodes [k*6272, (k+1)*6272) = 49 windows, processed in a
  per-core load-balancing slot order (windows sorted by edge count desc so
  the shared SPMD tile counts T[s] = max over cores are tight).
- Phase A (node-sharded): h0=relu(x@W1.T+b1), ego=LN(h0) (rstd via
  exp(-0.5*ln(var+eps)) so ScalarE stays on one act table), h=ego@Wlin.T
  per slot; h shard packed PAIRED into hown [SH/2, 128] rows
  [h_p | h_{p+64}] (two contiguous-partition DMAs), AllGather ->
  hgall [NP/2, 128] (6.4MB vs 12.8MB unpaired).
- Phase B per window: edges split by col parity ((col%128)//64) into E/O
  streams (gather idx = paired row < 25088, int16-safe, no 32768 split);
  gpsimd dma_gather streams fetch h_col rows in CALL-row chunks (parity
  selects col half). Per tile: gather-mm (streamed ST05 one-hot lhsT) +
  ident-mm build tt in grouped PSUM [P, G*64]; one batched DVE
  STT(relu*wd) + grouped tensor_reduce -> d; att = 1/(1+exp(-d)) (Exp on
  ScalarE, add+reciprocal on DVE); xj = att*hc batched per chunk-run via
  broadcast AP; scatter via PE matmuls with S built ON-CHIP (one batched
  DVE is_equal per window vs iota). agg half1 = sum(hc) - half0.
- Phase C: relu(agg), LN (ln/exp), blend with ego (0.5 into W2), GEMM W2;
  outputs staged in SBUF, one DMA; host unpermutes window slots.
All ScalarE funcs in {Relu, Copy, Ln, Exp} -> single act table load.
"""
import os as _os_mod

import numpy as np

N = 50000
E = 800000
IN = 128
HID = 64
C = 2
HC = 128
OUT = 40
BETA = 0.5
TEMP = 1.0
EPS = 1e-5

NCORES = 8
P = 128
NP = 50176            # 392 tiles of 128
SH = NP // NCORES     # 6272 nodes/core, 49 windows
NWIN = SH // P        # 49
SHROW = SH // 2       # paired rows per core (3136)

CALL = int(_os_mod.environ.get("KCALL", "1024"))  # gather rows per call
CT = CALL // P
SCH = 8               # st05 tiles per streamed chunk
GRP = 8               # tt tiles per PSUM group

_cache = {}


def _bf16():
    import ml_dtypes

    return ml_dtypes.bfloat16


def _host_prep(x, edge_index, W1, b1, Wlin, Watt, W2, b2):
    bf16 = _bf16()
    x = np.asarray(x, np.float32)
    row = np.asarray(edge_index[0], np.int64)
    col = np.asarray(edge_index[1], np.int64)

    x_pad = np.zeros((NP, IN), np.float32)
    x_pad[:N] = x

    wglob = row // P                      # dest window 0..391
    wcount = np.bincount(wglob, minlength=NP // P)
    # per-core slot permutation: windows by edge count desc
    perms = []                            # perms[k][s] = global window
    inv_slot = np.zeros(NP // P, np.int64)
    for k in range(NCORES):
        wins = np.arange(k * NWIN, (k + 1) * NWIN)
        order = wins[np.argsort(-wcount[wins], kind="stable")]
        perms.append(order)
        inv_slot[order] = np.arange(NWIN)

    # gather row + parity for cols (remapped, paired (p, p+64))
    p_c = col % P
    w_c = col // P
    k_c = w_c // NWIN
    grow = (k_c * SHROW + inv_slot[w_c] * 64 + (p_c % 64)).astype(np.int64)
    par = (p_c // 64).astype(np.int64)
    rd = (row % P).astype(np.int64)
    core = wglob // NWIN
    slot = inv_slot[wglob]

    # bucket edges per (core, slot, parity), sorted by rd
    buckets = {}
    for k in range(NCORES):
        mk = core == k
        sk, pk, gk, rk = slot[mk], par[mk], grow[mk], rd[mk]
        for s in range(NWIN):
            ms = sk == s
            for pa in range(2):
                m = ms & (pk == pa)
                g, r = gk[m], rk[m]
                o = np.argsort(r, kind="stable")
                buckets[(k, s, pa)] = (g[o], r[o])

    T_E = np.zeros(NWIN, np.int64)
    T_O = np.zeros(NWIN, np.int64)
    for s in range(NWIN):
        T_E[s] = max(-(-len(buckets[(k, s, 0)][0]) // P) for k in range(NCORES))
        T_O[s] = max(-(-len(buckets[(k, s, 1)][0]) // P) for k in range(NCORES))
    MAXW = int(max(T_E[s] + T_O[s] for s in range(NWIN)))
    SE = int(T_E.sum()) * P
    SO = int(T_O.sum()) * P
    NT = int(T_E.sum() + T_O.sum())

    def wrap16(a):
        n = len(a)
        pad = (-n) % 16
        a = np.concatenate([a, np.zeros(pad, np.int16)])
        return np.tile(a.reshape(-1, 16).T, (8, 1))

    ar = np.arange(P)
    in_maps = []
    for k in range(NCORES):
        colE = np.zeros(SE, np.int16)
        colO = np.zeros(SO, np.int16)
        rd_all = np.full(NT * P, 200.0, np.float32)
        oe = oo = 0
        gt = 0
        for s in range(NWIN):
            (ge, re) = buckets[(k, s, 0)]
            (go, ro) = buckets[(k, s, 1)]
            ne, no = len(ge), len(go)
            colE[oe : oe + ne] = ge.astype(np.int16)
            colO[oo : oo + no] = go.astype(np.int16)
            rd_all[gt * P : gt * P + ne] = re.astype(np.float32)
            gt += int(T_E[s])
            rd_all[gt * P : gt * P + no] = ro.astype(np.float32)
            gt += int(T_O[s])
            oe += int(T_E[s]) * P
            oo += int(T_O[s]) * P

        # host-built ST05 [d, e] = 0.5*(d == rd[e]) per tile -> [P, NT*P]
        import ml_dtypes as _mld
        f8 = _mld.float8_e4m3
        rdm = rd_all.reshape(NT, P)
        st05 = ((ar[:, None, None] == rdm[None, :, :]).astype(f8)
                * f8(0.5)).reshape(P, NT * P)

        xk = np.empty((SH, IN), np.float32)
        for s in range(NWIN):
            w = perms[k][s]
            xk[s * P : (s + 1) * P] = x_pad[w * P : (w + 1) * P]
        in_maps.append(
            {
                "xT": np.ascontiguousarray(xk.T.astype(bf16)),
                "colE": wrap16(colE),
                "colO": wrap16(colO),
                "rdt": np.ascontiguousarray(
                    rd_all.reshape(NT, P).T.astype(bf16)
                ),
                "sarr": np.ascontiguousarray(st05),
            }
        )
    wd = (np.asarray(Watt[0]) - np.asarray(Watt[1])).astype(np.float32)
    shared = {
        "w1t": np.asarray(W1, np.float32).T.astype(bf16).copy(),
        "b1row": np.asarray(b1, np.float32)[None, :].astype(bf16),
        "wlint": np.asarray(Wlin, np.float32).T.astype(bf16).copy(),
        "wdrep": np.tile(wd[None, :], (P, 1)).astype(bf16),
        "iotac": np.tile(
            np.arange(P, dtype=np.float32)[None, :], (P, 1)
        ).astype(bf16),
        "iotar": np.tile(
            np.repeat(np.arange(P, dtype=np.float32), MAXW)[None, :], (P, 1)
        ).astype(bf16),
        "w2t": ((1.0 - BETA) * np.asarray(W2, np.float32).T).astype(bf16).copy(),
        "b2row": np.asarray(b2, np.float32)[None, :].astype(bf16),
    }
    for im in in_maps:
        im.update(shared)
    return in_maps, (tuple(T_E.tolist()), tuple(T_O.tolist())), perms


def _build(T_E, T_O, reps=1):
    import concourse.bacc as bacc
    import concourse.mybir as mybir
    import concourse.tile as tile
    from concourse.library_config import mlp
    from concourse.masks import make_identity

    f32 = mybir.dt.float32
    bf = mybir.dt.bfloat16
    i16 = mybir.dt.int16
    Alu = mybir.AluOpType
    Act = mybir.ActivationFunctionType

    SE = sum(T_E) * P
    SO = sum(T_O) * P
    NT = sum(T_E) + sum(T_O)
    MAXW = max(te + to for te, to in zip(T_E, T_O))

    NSWQ = int(_os_mod.environ.get("KSWQ", "2"))
    KSCR = int(_os_mod.environ.get("KSCR", "16384"))
    KSTAGE = int(_os_mod.environ.get("KSTAGE", "0"))  # 0=full bisect gates
    nc = bacc.Bacc("TRN2", num_devices=NCORES, num_swdge_queues=NSWQ,
                   dynamic_dma_scratch_size=KSCR)
    xT = nc.dram_tensor("xT", [IN, SH], bf, kind="ExternalInput")
    colE = nc.dram_tensor("colE", [P, (SE + 15) // 16], i16, kind="ExternalInput")
    colO = nc.dram_tensor("colO", [P, (SO + 15) // 16], i16, kind="ExternalInput")
    rdt = nc.dram_tensor("rdt", [P, NT], bf, kind="ExternalInput")
    f8 = mybir.dt.float8e4
    sarr = nc.dram_tensor("sarr", [P, NT * P], f8, kind="ExternalInput")
    w1t = nc.dram_tensor("w1t", [IN, HC], bf, kind="ExternalInput")
    b1row = nc.dram_tensor("b1row", [1, HC], bf, kind="ExternalInput")
    wlint = nc.dram_tensor("wlint", [HC, HID], bf, kind="ExternalInput")
    wdrep = nc.dram_tensor("wdrep", [P, HID], bf, kind="ExternalInput")
    iotac = nc.dram_tensor("iotac", [P, P], bf, kind="ExternalInput")
    iotar = nc.dram_tensor("iotar", [P, P * MAXW], bf, kind="ExternalInput")
    w2t = nc.dram_tensor("w2t", [HC, OUT], bf, kind="ExternalInput")
    b2row = nc.dram_tensor("b2row", [1, OUT], bf, kind="ExternalInput")
    hown = nc.dram_tensor("hown", [SHROW, P], bf)   # paired local shard
    hgall = nc.dram_tensor("hgall", [NP // 2, P], bf)  # allgathered table
    outd = nc.dram_tensor("out", [SH, OUT], f32, kind="ExternalOutput")
    import os as _os

    with tile.TileContext(nc) as tc:
        with (
            tc.tile_pool(name="const", bufs=1) as cp,
            tc.tile_pool(name="work", bufs=6) as wp,
            tc.tile_pool(name="gE", bufs=4) as gpe,
            tc.tile_pool(name="gO", bufs=4) as gpo,
            tc.tile_pool(name="sp", bufs=6) as spp,
            tc.tile_pool(name="sall", bufs=2) as sap,
            tc.tile_pool(name="xjp", bufs=2) as xjp,
            tc.tile_pool(name="pstr", bufs=1, space="PSUM") as pstr,
            tc.tile_pool(name="ptg", bufs=2, space="PSUM") as ptgp,
            tc.tile_pool(name="acc", bufs=2, space="PSUM") as accp,
        ):
            nc.gpsimd.load_library(mlp)
            # ---- constants to SBUF ----
            w1t_sb = cp.tile([IN, HC], bf, tag="w1t")
            b1_sb = cp.tile([1, HC], bf, tag="b1")
            wlint_sb = cp.tile([HC, HID], bf, tag="wlt")
            wd_sb = cp.tile([P, HID], bf, tag="wd")
            iota_sb = cp.tile([P, P], bf, tag="iota")
            iotar_sb = cp.tile([P, P * MAXW], bf, tag="iotar")
            w2t_sb = cp.tile([HC, OUT], bf, tag="w2t")
            b2_sb = cp.tile([1, OUT], bf, tag="b2")
            colE_sb = cp.tile([P, (SE + 15) // 16], i16, tag="colE")
            colO_sb = cp.tile([P, (SO + 15) // 16], i16, tag="colO")
            rdt_sb = cp.tile([P, NT], bf, tag="rdt")
            xT_sb = cp.tile([IN, SH], bf, tag="xT")
            for sb, dr in (
                (w1t_sb, w1t), (b1_sb, b1row), (wlint_sb, wlint),
                (wd_sb, wdrep), (iota_sb, iotac), (iotar_sb, iotar),
                (w2t_sb, w2t),
                (b2_sb, b2row), (colE_sb, colE), (colO_sb, colO),
                (rdt_sb, rdt), (xT_sb, xT),
            ):
                nc.sync.dma_start(sb[:], dr[:])
            ident = cp.tile([P, P], bf, tag="ident")
            make_identity(nc, ident[:])
            ones1 = cp.tile([1, P], bf, tag="ones1")
            nc.vector.memset(ones1[:], 1.0)
            eps_sb = cp.tile([P, 1], f32, tag="eps")
            nc.vector.memset(eps_sb[:], EPS)
            ego_sb = cp.tile([P, NWIN, HC], bf, tag="ego")
            agg_sb = cp.tile([P, NWIN, HC], bf, tag="agg")
            hall_sb = cp.tile([P, NWIN, HID], bf, tag="hall")
            o_sb = cp.tile([P, NWIN, OUT], f32, tag="osb")

            for rep in range(reps):
                tc.strict_bb_all_engine_barrier()
                # ================= Phase A =================
                for gt in range(NWIN):
                    psAt = ptgp.tile([P, GRP * HID], f32, tag="ptg")
                    psA = psAt[:, 0:HC]
                    nc.tensor.matmul(out=psA, lhsT=xT_sb[:, gt * P : (gt + 1) * P],
                                     rhs=w1t_sb[:], start=True, stop=False)
                    nc.tensor.matmul(out=psA, lhsT=ones1[:], rhs=b1_sb[:],
                                     start=False, stop=True)
                    r = wp.tile([P, HC], bf, tag="r")
                    rsum = wp.tile([P, 1], f32, tag="rsum")
                    nc.scalar.activation(r[:], psA, Act.Relu, accum_out=rsum[:])
                    negmu = wp.tile([P, 1], f32, tag="negmu")
                    nc.vector.tensor_scalar(out=negmu[:], in0=rsum[:],
                                            scalar1=-1.0 / HC, scalar2=None,
                                            op0=Alu.mult)
                    cen = wp.tile([P, HC], bf, tag="cen")
                    nc.scalar.activation(cen[:], r[:], Act.Identity,
                                         bias=negmu[:])
                    vsum = wp.tile([P, 1], f32, tag="vsum")
                    junk = wp.tile([P, HC], bf, tag="junkA")
                    nc.vector.scalar_tensor_tensor(
                        out=junk[:], in0=cen[:], scalar=1.0, in1=cen[:],
                        op0=Alu.mult, op1=Alu.mult, accum_out=vsum[:])
                    sd = wp.tile([P, 1], f32, tag="sd")
                    nc.scalar.activation(sd[:], vsum[:], Act.Sqrt,
                                         bias=eps_sb[:], scale=1.0 / HC)
                    rstd = wp.tile([P, 1], f32, tag="rstd")
                    nc.vector.reciprocal(rstd[:], sd[:])
                    nc.scalar.activation(ego_sb[:, gt, :], cen[:], Act.Copy,
                                         scale=rstd[:])
                    egoT_ps = pstr.tile([P, HC], bf, tag="ptr")
                    nc.tensor.transpose(out=egoT_ps[:], in_=ego_sb[:, gt, :],
                                        identity=ident[:])
                    egoT_sb = wp.tile([HC, P], bf, tag="egoT")
                    nc.scalar.activation(egoT_sb[:], egoT_ps[:], Act.Copy)
                    hpst = ptgp.tile([P, GRP * HID], f32, tag="ptg")
                    nc.tensor.matmul(out=hpst[:, 0:HID], lhsT=egoT_sb[:],
                                     rhs=wlint_sb[:], start=True, stop=True)
                    nc.scalar.activation(hall_sb[:, gt, :], hpst[:, 0:HID],
                                         Act.Copy)
                # paired h shard -> DRAM (two contiguous-half DMAs)
                hr = hown[:].rearrange("(s p2) f -> p2 s f", p2=64)
                nc.sync.dma_start(hr[:, :, 0:HID], hall_sb[0:64, :, :])
                nc.sync.dma_start(hr[:, :, HID:P], hall_sb[64:P, :, :])
                if _os.environ.get("KCCBAR"):
                    tc.strict_bb_all_engine_barrier()
                if not _os.environ.get("KSIM_NOCC"):
                    nc.gpsimd.collective_compute(
                        "AllGather",
                        mybir.AluOpType.bypass,
                        replica_groups=[list(range(NCORES))],
                        ins=[hown[:].opt()],
                        outs=[hgall[:].opt()],
                    )
                if _os.environ.get("KCCBAR"):
                    tc.strict_bb_all_engine_barrier()

                # ================= Phase B =================
                chunks = {"E": {}, "O": {}}
                streams = {
                    "E": (colE_sb, SE, gpe, 0, 0),
                    "O": (colO_sb, SO, gpo, 1, HID),
                }

                def get_tile(stream, g):
                    """-> (chunk_tile, sub, p0): hc = chunk[:, sub, p0:p0+HID]"""
                    colsb, stot, pool, q, p0 = streams[stream]
                    c = g * P // CALL
                    sub = (g * P % CALL) // P
                    bufs = chunks[stream]
                    if c not in bufs:
                        n_i = min(CALL, stot - c * CALL)
                        n6 = n_i // P
                        hcb = pool.tile([P, CT, P], bf, tag="hc" + stream)
                        i0 = c * (CALL // 16)
                        i1 = i0 + (n_i + 15) // 16
                        if _os.environ.get("KNOGATHER"):
                            nc.sync.dma_start(
                                hcb[:, :n6, :],
                                hgall[0 : n6 * P, :].rearrange(
                                    "(t p) f -> p t f", p=P))
                        else:
                            nc.gpsimd.dma_gather(
                                hcb[:, :n6, :], hgall[:], colsb[:, i0:i1],
                                n_i, n_i, P, queue_num=q % NSWQ)
                        bufs[c] = hcb
                    return bufs[c], sub, p0

                schunks = {}

                def get_st05(gtile):
                    cs = gtile // SCH
                    if cs not in schunks:
                        n_t = min(SCH, NT - cs * SCH)
                        sb_ = spp.tile([P, SCH, P], f8, tag="sch")
                        if not _os.environ.get("KNOST05"):
                            nc.sync.dma_start(
                                sb_[:, :n_t, :],
                                sarr[:, cs * SCH * P : (cs * SCH + n_t) * P],
                            )
                        schunks[cs] = sb_
                    return schunks[cs][:, gtile % SCH, :]

                gcnt = {"E": 0, "O": 0}
                gt = 0
                for wi in range(NWIN):
                    ntile = T_E[wi] + T_O[wi]
                    if ntile == 0 or KSTAGE == 1:
                        nc.vector.memset(agg_sb[:, wi, :], 0.0)
                        continue
                    plan = []  # (stream, g, gtile)
                    g0 = {"E": gcnt["E"], "O": gcnt["O"]}
                    for stream, tcount in (("E", T_E[wi]), ("O", T_O[wi])):
                        for _ in range(tcount):
                            plan.append((stream, gcnt[stream], gt))
                            gcnt[stream] += 1
                            gt += 1
                    gt0 = gt - ntile

                    # on-chip S build, [e, d, t] layout (packed innermost
                    # on both operands -> DVE 2x mode eligible)
                    sall = sap.tile([P, P, MAXW], bf, tag="sall")
                    rdt_b = rdt_sb[:, gt0:gt0 + ntile].rearrange(
                        "p (o t) -> p o t", o=1).broadcast_to([P, P, ntile])
                    iot_b = iotar_sb[:].rearrange(
                        "p (d t) -> p d t", t=MAXW)[:, :, 0:ntile]
                    nc.vector.tensor_tensor(out=sall[:, :, 0:ntile],
                                            in0=rdt_b, in1=iot_b,
                                            op=Alu.is_equal)
                    if KSTAGE == 2:
                        for stream, tcount in (("E", T_E[wi]), ("O", T_O[wi])):
                            for j in range(tcount):
                                pass
                        for j in range(ntile):
                            get_tile(*plan[j][:2])
                            get_st05(gt0 + j)
                        nc.vector.memset(agg_sb[:, wi, :], 0.0)
                        for stream in ("E", "O"):
                            done = (g0[stream] * P) // CALL
                            for c in [c for c in chunks[stream] if c < done]:
                                del chunks[stream][c]
                        sdone = gt0 // SCH
                        for c in [c for c in schunks if c < sdone]:
                            del schunks[c]
                        continue

                    # tt in grouped PSUM; d per group via STT + reduce
                    ddwin = wp.tile([P, MAXW], f32, tag="ddwin")
                    ngrp = -(-ntile // GRP)
                    for gi in range(ngrp):
                        a = gi * GRP
                        g = min(GRP, ntile - a)
                        ptt = ptgp.tile([P, GRP * HID], f32, tag="ptg")
                        for j in range(g):
                            stream, gg, _ = plan[a + j]
                            hcb, sub, p0 = get_tile(stream, gg)
                            st05 = get_st05(gt0 + a + j)
                            nc.tensor.matmul(out=ptt[:, j * HID:(j + 1) * HID],
                                             lhsT=st05,
                                             rhs=hall_sb[:, wi, :],
                                             start=True, stop=False)
                            nc.tensor.matmul(out=ptt[:, j * HID:(j + 1) * HID],
                                             lhsT=ident[:],
                                             rhs=hcb[:, sub, p0:p0 + HID],
                                             start=False, stop=True)
                        rtt = wp.tile([P, GRP * HID], bf, tag="rtt")
                        wd_b = wd_sb[:].rearrange(
                            "p (o f) -> p o f", o=1).broadcast_to([P, g, HID])
                        nc.vector.scalar_tensor_tensor(
                            out=rtt[:].rearrange("p (t f) -> p t f", t=GRP)[:, 0:g, :],
                            in0=ptt[:].rearrange("p (t f) -> p t f", t=GRP)[:, 0:g, :],
                            scalar=0.0, in1=wd_b, op0=Alu.max, op1=Alu.mult)
                        nc.vector.tensor_reduce(
                            out=ddwin[:, a:a + g],
                            in_=rtt[:].rearrange("p (t f) -> p t f", t=GRP)[:, 0:g, :],
                            axis=mybir.AxisListType.X, op=Alu.add)

                    # att = sigmoid(d)
                    attw = wp.tile([P, MAXW], bf, tag="attw")
                    nc.scalar.activation(attw[:, 0:ntile], ddwin[:, 0:ntile],
                                         Act.Sigmoid)
                    if KSTAGE == 3:
                        nc.vector.memset(agg_sb[:, wi, :], 0.0)
                        for stream in ("E", "O"):
                            done = (g0[stream] * P) // CALL
                            for c in [c for c in chunks[stream] if c < done]:
                                del chunks[stream][c]
                        sdone = gt0 // SCH
                        for c in [c for c in schunks if c < sdone]:
                            del schunks[c]
                        continue

                    # xj batched per (stream, chunk) run
                    xjw = xjp.tile([P, MAXW, HID], bf, tag="xjw")
                    if _os.environ.get("KXJTILE"):
                        for ti in range(ntile):
                            stream, gg, _ = plan[ti]
                            hcb, sub, p0 = get_tile(stream, gg)
                            nc.vector.tensor_scalar(
                                out=xjw[:, ti, :],
                                in0=hcb[:, sub, p0:p0 + HID],
                                scalar1=attw[:, ti:ti + 1], scalar2=None,
                                op0=Alu.mult)
                    else:
                        ti = 0
                        while ti < ntile:
                            stream, gg, _ = plan[ti]
                            hcb, sub, p0 = get_tile(stream, gg)
                            L = 1
                            while (ti + L < ntile and plan[ti + L][0] == stream
                                   and plan[ti + L][1] == gg + L
                                   and sub + L < CT + 1 and (gg + L) * P // CALL
                                   == gg * P // CALL):
                                L += 1
                            att_b = attw[:, ti:ti + L].rearrange(
                                "p (t o) -> p t o", o=1).broadcast_to([P, L, HID])
                            nc.vector.tensor_tensor(
                                out=xjw[:, ti:ti + L, :],
                                in0=hcb[:, sub:sub + L, p0:p0 + HID],
                                in1=att_b, op=Alu.mult)
                            ti += L

                    # scatter (acc0/acc1 in separate PSUM banks)
                    acc0 = accp.tile([P, HID], f32, tag="acc0")
                    acc1 = accp.tile([P, HID], f32, tag="acc1")
                    for ti, (stream, gg, _) in enumerate(plan):
                        hcb, sub, p0 = get_tile(stream, gg)
                        st = ti == 0
                        sp = ti == ntile - 1
                        nc.tensor.matmul(out=acc0[:], lhsT=sall[:, :, ti],
                                         rhs=xjw[:, ti, :], start=st, stop=sp)
                        nc.tensor.matmul(out=acc1[:], lhsT=sall[:, :, ti],
                                         rhs=hcb[:, sub, p0:p0 + HID],
                                         start=st, stop=sp)
                    nc.scalar.activation(agg_sb[:, wi, 0:HID], acc0[:],
                                         Act.Copy)
                    with nc.allow_low_precision(reason="agg bf16"):
                        nc.vector.tensor_tensor(
                            out=agg_sb[:, wi, HID:HC], in0=acc1[:],
                            in1=agg_sb[:, wi, 0:HID], op=Alu.subtract)
                    # free consumed chunks
                    for stream in ("E", "O"):
                        done = (g0[stream] * P) // CALL
                        for c in [c for c in chunks[stream] if c < done]:
                            del chunks[stream][c]
                    sdone = gt0 // SCH
                    for c in [c for c in schunks if c < sdone]:
                        del schunks[c]

                # ================= Phase C =================
                # barrier keeps Phase C's Sqrt table swaps out of Phase B
                tc.strict_bb_all_engine_barrier()
                for wi in range(NWIN):
                    xh = wp.tile([P, HC], bf, tag="xh")
                    rsum = wp.tile([P, 1], f32, tag="rsum")
                    nc.scalar.activation(xh[:], agg_sb[:, wi, :], Act.Relu,
                                         accum_out=rsum[:])
                    negmu = wp.tile([P, 1], f32, tag="negmu")
                    nc.vector.tensor_scalar(out=negmu[:], in0=rsum[:],
                                            scalar1=-1.0 / HC, scalar2=None,
                                            op0=Alu.mult)
                    cen = wp.tile([P, HC], bf, tag="cen")
                    nc.scalar.activation(cen[:], xh[:], Act.Identity,
                                         bias=negmu[:])
                    vsum = wp.tile([P, 1], f32, tag="vsum")
                    junk = wp.tile([P, HC], bf, tag="junkA")
                    nc.vector.scalar_tensor_tensor(
                        out=junk[:], in0=cen[:], scalar=1.0, in1=cen[:],
                        op0=Alu.mult, op1=Alu.mult, accum_out=vsum[:])
                    sd = wp.tile([P, 1], f32, tag="sd")
                    nc.scalar.activation(sd[:], vsum[:], Act.Sqrt,
                                         bias=eps_sb[:], scale=1.0 / HC)
                    rstd = wp.tile([P, 1], f32, tag="rstd")
                    nc.vector.reciprocal(rstd[:], sd[:])
                    ln = wp.tile([P, HC], bf, tag="ln")
                    nc.scalar.activation(ln[:], cen[:], Act.Copy,
                                         scale=rstd[:])
                    xb = wp.tile([P, HC], bf, tag="xb")
                    with nc.allow_low_precision(reason="blend bf16"):
                        nc.vector.tensor_tensor(out=xb[:], in0=ln[:],
                                                in1=ego_sb[:, wi, :], op=Alu.add)
                    xbT_ps = pstr.tile([P, HC], bf, tag="ptr")
                    nc.tensor.transpose(out=xbT_ps[:], in_=xb[:], identity=ident[:])
                    xbT_sb = wp.tile([HC, P], bf, tag="xbT")
                    nc.scalar.activation(xbT_sb[:], xbT_ps[:], Act.Copy)
                    psOt = ptgp.tile([P, GRP * HID], f32, tag="ptg")
                    nc.tensor.matmul(out=psOt[:, 0:OUT], lhsT=xbT_sb[:],
                                     rhs=w2t_sb[:], start=True, stop=False)
                    nc.tensor.matmul(out=psOt[:, 0:OUT], lhsT=ones1[:],
                                     rhs=b2_sb[:], start=False, stop=True)
                    nc.vector.tensor_copy(o_sb[:, wi, :], psOt[:, 0:OUT])
                nc.sync.dma_start(
                    outd[:].rearrange("(t p) f -> p t f", p=P), o_sb[:]
                )
    nc.compile()
    return nc


def _get_compiled(key, T_E, T_O, reps):
    if key not in _cache:
        _cache[key] = _build(T_E, T_O, reps)
    return _cache[key]


def prepare(inputs, reps=1):
    """Host prep + build; returns (nc, in_maps, perms)."""
    g0 = np.asarray(inputs["g0"])
    beta0 = np.asarray(inputs["beta0"])
    g1 = np.asarray(inputs["g1"])
    beta1 = np.asarray(inputs["beta1"])
    assert np.allclose(g0, 1.0) and np.allclose(beta0, 0.0)
    assert np.allclose(g1, 1.0) and np.allclose(beta1, 0.0)
    in_maps, (T_E, T_O), perms = _host_prep(
        inputs["x"], inputs["edge_index"], inputs["W1"], inputs["b1"],
        inputs["Wlin"], inputs["Watt"], inputs["W2"], inputs["b2"],
    )
    key = (T_E, T_O, reps)
    nc = _get_compiled(key, list(T_E), list(T_O), reps)
    return nc, in_maps, perms


def kernel(**inputs) -> np.ndarray:
    from concourse.bass_utils import run_bass_kernel_spmd

    nc, in_maps, perms = prepare(inputs, reps=1)
    res = run_bass_kernel_spmd(nc, in_maps, list(range(NCORES)))
    full = np.empty((NP, OUT), np.float32)
    for k in range(NCORES):
        ok = res.results[k]["out"]          # [SH, OUT] slot-ordered
        for s in range(NWIN):
            w = perms[k][s]
            full[w * P : (w + 1) * P] = ok[s * P : (s + 1) * P]
    return full[:N]


# revision 36
# speedup vs baseline: 1.0551x; 1.0551x over previous
"""M2M-GNN (nn_M2MGNNPro) Trainium2 kernel, 8-core SPMD, bf16. v3.

Strategy (edge-parallel, destination-sharded, slot-permuted windows):
- Core k owns dest nodes [k*6272, (k+1)*6272) = 49 windows, processed in a
  per-core load-balancing slot order (windows sorted by edge count desc so
  the shared SPMD tile counts T[s] = max over cores are tight).
- Phase A (node-sharded): h0=relu(x@W1.T+b1), ego=LN(h0) (rstd via
  exp(-0.5*ln(var+eps)) so ScalarE stays on one act table), h=ego@Wlin.T
  per slot; h shard packed PAIRED into hown [SH/2, 128] rows
  [h_p | h_{p+64}] (two contiguous-partition DMAs), AllGather ->
  hgall [NP/2, 128] (6.4MB vs 12.8MB unpaired).
- Phase B per window: edges split by col parity ((col%128)//64) into E/O
  streams (gather idx = paired row < 25088, int16-safe, no 32768 split);
  gpsimd dma_gather streams fetch h_col rows in CALL-row chunks (parity
  selects col half). Per tile: gather-mm (streamed ST05 one-hot lhsT) +
  ident-mm build tt in grouped PSUM [P, G*64]; one batched DVE
  STT(relu*wd) + grouped tensor_reduce -> d; att = 1/(1+exp(-d)) (Exp on
  ScalarE, add+reciprocal on DVE); xj = att*hc batched per chunk-run via
  broadcast AP; scatter via PE matmuls with S built ON-CHIP (one batched
  DVE is_equal per window vs iota). agg half1 = sum(hc) - half0.
- Phase C: relu(agg), LN (ln/exp), blend with ego (0.5 into W2), GEMM W2;
  outputs staged in SBUF, one DMA; host unpermutes window slots.
All ScalarE funcs in {Relu, Copy, Ln, Exp} -> single act table load.
"""
import os as _os_mod

import numpy as np

N = 50000
E = 800000
IN = 128
HID = 64
C = 2
HC = 128
OUT = 40
BETA = 0.5
TEMP = 1.0
EPS = 1e-5

NCORES = 8
P = 128
NP = 50176            # 392 tiles of 128
SH = NP // NCORES     # 6272 nodes/core, 49 windows
NWIN = SH // P        # 49
SHROW = SH // 2       # paired rows per core (3136)

CALL = int(_os_mod.environ.get("KCALL", "1024"))  # gather rows per call
CT = CALL // P
SCH = 16              # st05 tiles per streamed chunk
GRP = 8               # tt tiles per PSUM group

_cache = {}


def _bf16():
    import ml_dtypes

    return ml_dtypes.bfloat16


def _host_prep(x, edge_index, W1, b1, Wlin, Watt, W2, b2):
    bf16 = _bf16()
    x = np.asarray(x, np.float32)
    row = np.asarray(edge_index[0], np.int64)
    col = np.asarray(edge_index[1], np.int64)

    x_pad = np.zeros((NP, IN), np.float32)
    x_pad[:N] = x

    wglob = row // P                      # dest window 0..391
    wcount = np.bincount(wglob, minlength=NP // P)
    # per-core slot permutation: windows by edge count desc
    perms = []                            # perms[k][s] = global window
    inv_slot = np.zeros(NP // P, np.int64)
    for k in range(NCORES):
        wins = np.arange(k * NWIN, (k + 1) * NWIN)
        order = wins[np.argsort(-wcount[wins], kind="stable")]
        perms.append(order)
        inv_slot[order] = np.arange(NWIN)

    # gather row + parity for cols (remapped, paired (p, p+64))
    p_c = col % P
    w_c = col // P
    k_c = w_c // NWIN
    grow = (k_c * SHROW + inv_slot[w_c] * 64 + (p_c % 64)).astype(np.int64)
    par = (p_c // 64).astype(np.int64)
    rd = (row % P).astype(np.int64)
    core = wglob // NWIN
    slot = inv_slot[wglob]

    # bucket edges per (core, slot, parity), sorted by rd
    buckets = {}
    for k in range(NCORES):
        mk = core == k
        sk, pk, gk, rk = slot[mk], par[mk], grow[mk], rd[mk]
        for s in range(NWIN):
            ms = sk == s
            for pa in range(2):
                m = ms & (pk == pa)
                g, r = gk[m], rk[m]
                o = np.argsort(r, kind="stable")
                buckets[(k, s, pa)] = (g[o], r[o])

    T_E = np.zeros(NWIN, np.int64)
    T_O = np.zeros(NWIN, np.int64)
    for s in range(NWIN):
        T_E[s] = max(-(-len(buckets[(k, s, 0)][0]) // P) for k in range(NCORES))
        T_O[s] = max(-(-len(buckets[(k, s, 1)][0]) // P) for k in range(NCORES))
    MAXW = int(max(T_E[s] + T_O[s] for s in range(NWIN)))
    SE = int(T_E.sum()) * P
    SO = int(T_O.sum()) * P
    NT = int(T_E.sum() + T_O.sum())

    def wrap16(a):
        n = len(a)
        pad = (-n) % 16
        a = np.concatenate([a, np.zeros(pad, np.int16)])
        return np.tile(a.reshape(-1, 16).T, (8, 1))

    ar = np.arange(P)
    in_maps = []
    for k in range(NCORES):
        colE = np.zeros(SE, np.int16)
        colO = np.zeros(SO, np.int16)
        rd_all = np.full(NT * P, 200.0, np.float32)
        oe = oo = 0
        gt = 0
        for s in range(NWIN):
            (ge, re) = buckets[(k, s, 0)]
            (go, ro) = buckets[(k, s, 1)]
            ne, no = len(ge), len(go)
            colE[oe : oe + ne] = ge.astype(np.int16)
            colO[oo : oo + no] = go.astype(np.int16)
            rd_all[gt * P : gt * P + ne] = re.astype(np.float32)
            gt += int(T_E[s])
            rd_all[gt * P : gt * P + no] = ro.astype(np.float32)
            gt += int(T_O[s])
            oe += int(T_E[s]) * P
            oo += int(T_O[s]) * P

        # host-built ST05 [d, e] = 0.5*(d == rd[e]) per tile -> [P, NT*P]
        import ml_dtypes as _mld
        f8 = _mld.float8_e4m3
        rdm = rd_all.reshape(NT, P)
        st05 = ((ar[:, None, None] == rdm[None, :, :]).astype(f8)
                * f8(0.5)).reshape(P, NT * P)

        xk = np.empty((SH, IN), np.float32)
        for s in range(NWIN):
            w = perms[k][s]
            xk[s * P : (s + 1) * P] = x_pad[w * P : (w + 1) * P]
        in_maps.append(
            {
                "xT": np.ascontiguousarray(xk.T.astype(bf16)),
                "colE": wrap16(colE),
                "colO": wrap16(colO),
                "rdt": np.ascontiguousarray(
                    rd_all.reshape(NT, P).T.astype(bf16)
                ),
                "sarr": np.ascontiguousarray(st05),
            }
        )
    wd = (np.asarray(Watt[0]) - np.asarray(Watt[1])).astype(np.float32)
    shared = {
        "w1t": np.asarray(W1, np.float32).T.astype(bf16).copy(),
        "b1row": np.asarray(b1, np.float32)[None, :].astype(bf16),
        "wlint": np.asarray(Wlin, np.float32).T.astype(bf16).copy(),
        "wdrep": np.tile(wd[None, :], (P, 1)).astype(bf16),
        "iotac": np.tile(
            np.arange(P, dtype=np.float32)[None, :], (P, 1)
        ).astype(bf16),
        "iotar": np.tile(
            np.repeat(np.arange(P, dtype=np.float32), MAXW)[None, :], (P, 1)
        ).astype(bf16),
        "w2t": ((1.0 - BETA) * np.asarray(W2, np.float32).T).astype(bf16).copy(),
        "b2row": np.asarray(b2, np.float32)[None, :].astype(bf16),
    }
    for im in in_maps:
        im.update(shared)
    return in_maps, (tuple(T_E.tolist()), tuple(T_O.tolist())), perms


def _build(T_E, T_O, reps=1):
    import concourse.bacc as bacc
    import concourse.mybir as mybir
    import concourse.tile as tile
    from concourse.library_config import mlp
    from concourse.masks import make_identity

    f32 = mybir.dt.float32
    bf = mybir.dt.bfloat16
    i16 = mybir.dt.int16
    Alu = mybir.AluOpType
    Act = mybir.ActivationFunctionType

    SE = sum(T_E) * P
    SO = sum(T_O) * P
    NT = sum(T_E) + sum(T_O)
    MAXW = max(te + to for te, to in zip(T_E, T_O))

    NSWQ = int(_os_mod.environ.get("KSWQ", "2"))
    KSCR = int(_os_mod.environ.get("KSCR", "16384"))
    KSTAGE = int(_os_mod.environ.get("KSTAGE", "0"))  # 0=full bisect gates
    nc = bacc.Bacc("TRN2", num_devices=NCORES, num_swdge_queues=NSWQ,
                   dynamic_dma_scratch_size=KSCR)
    xT = nc.dram_tensor("xT", [IN, SH], bf, kind="ExternalInput")
    colE = nc.dram_tensor("colE", [P, (SE + 15) // 16], i16, kind="ExternalInput")
    colO = nc.dram_tensor("colO", [P, (SO + 15) // 16], i16, kind="ExternalInput")
    rdt = nc.dram_tensor("rdt", [P, NT], bf, kind="ExternalInput")
    f8 = mybir.dt.float8e4
    sarr = nc.dram_tensor("sarr", [P, NT * P], f8, kind="ExternalInput")
    w1t = nc.dram_tensor("w1t", [IN, HC], bf, kind="ExternalInput")
    b1row = nc.dram_tensor("b1row", [1, HC], bf, kind="ExternalInput")
    wlint = nc.dram_tensor("wlint", [HC, HID], bf, kind="ExternalInput")
    wdrep = nc.dram_tensor("wdrep", [P, HID], bf, kind="ExternalInput")
    iotac = nc.dram_tensor("iotac", [P, P], bf, kind="ExternalInput")
    iotar = nc.dram_tensor("iotar", [P, P * MAXW], bf, kind="ExternalInput")
    w2t = nc.dram_tensor("w2t", [HC, OUT], bf, kind="ExternalInput")
    b2row = nc.dram_tensor("b2row", [1, OUT], bf, kind="ExternalInput")
    hown = nc.dram_tensor("hown", [SHROW, P], bf)   # paired local shard
    hgall = nc.dram_tensor("hgall", [NP // 2, P], bf)  # allgathered table
    outd = nc.dram_tensor("out", [SH, OUT], f32, kind="ExternalOutput")
    import os as _os

    with tile.TileContext(nc) as tc:
        with (
            tc.tile_pool(name="const", bufs=1) as cp,
            tc.tile_pool(name="work", bufs=6) as wp,
            tc.tile_pool(name="gE", bufs=4) as gpe,
            tc.tile_pool(name="gO", bufs=4) as gpo,
            tc.tile_pool(name="sp", bufs=6) as spp,
            tc.tile_pool(name="sall", bufs=2) as sap,
            tc.tile_pool(name="xjp", bufs=2) as xjp,
            tc.tile_pool(name="pstr", bufs=1, space="PSUM") as pstr,
            tc.tile_pool(name="ptg", bufs=2, space="PSUM") as ptgp,
            tc.tile_pool(name="acc", bufs=2, space="PSUM") as accp,
        ):
            nc.gpsimd.load_library(mlp)
            # ---- constants to SBUF ----
            w1t_sb = cp.tile([IN, HC], bf, tag="w1t")
            b1_sb = cp.tile([1, HC], bf, tag="b1")
            wlint_sb = cp.tile([HC, HID], bf, tag="wlt")
            wd_sb = cp.tile([P, HID], bf, tag="wd")
            iota_sb = cp.tile([P, P], bf, tag="iota")
            iotar_sb = cp.tile([P, P * MAXW], bf, tag="iotar")
            w2t_sb = cp.tile([HC, OUT], bf, tag="w2t")
            b2_sb = cp.tile([1, OUT], bf, tag="b2")
            colE_sb = cp.tile([P, (SE + 15) // 16], i16, tag="colE")
            colO_sb = cp.tile([P, (SO + 15) // 16], i16, tag="colO")
            rdt_sb = cp.tile([P, NT], bf, tag="rdt")
            xT_sb = cp.tile([IN, SH], bf, tag="xT")
            for sb, dr in (
                (w1t_sb, w1t), (b1_sb, b1row), (wlint_sb, wlint),
                (wd_sb, wdrep), (iota_sb, iotac), (iotar_sb, iotar),
                (w2t_sb, w2t),
                (b2_sb, b2row), (colE_sb, colE), (colO_sb, colO),
                (rdt_sb, rdt), (xT_sb, xT),
            ):
                nc.sync.dma_start(sb[:], dr[:])
            ident = cp.tile([P, P], bf, tag="ident")
            make_identity(nc, ident[:])
            ones1 = cp.tile([1, P], bf, tag="ones1")
            nc.vector.memset(ones1[:], 1.0)
            eps_sb = cp.tile([P, 1], f32, tag="eps")
            nc.vector.memset(eps_sb[:], EPS)
            ego_sb = cp.tile([P, NWIN, HC], bf, tag="ego")
            agg_sb = cp.tile([P, NWIN, HC], bf, tag="agg")
            hall_sb = cp.tile([P, NWIN, HID], bf, tag="hall")
            o_sb = cp.tile([P, NWIN, OUT], f32, tag="osb")

            for rep in range(reps):
                tc.strict_bb_all_engine_barrier()
                # ================= Phase A =================
                for gt in range(NWIN):
                    psAt = ptgp.tile([P, GRP * HID], f32, tag="ptg")
                    psA = psAt[:, 0:HC]
                    nc.tensor.matmul(out=psA, lhsT=xT_sb[:, gt * P : (gt + 1) * P],
                                     rhs=w1t_sb[:], start=True, stop=False)
                    nc.tensor.matmul(out=psA, lhsT=ones1[:], rhs=b1_sb[:],
                                     start=False, stop=True)
                    r = wp.tile([P, HC], bf, tag="r")
                    rsum = wp.tile([P, 1], f32, tag="rsum")
                    nc.scalar.activation(r[:], psA, Act.Relu, accum_out=rsum[:])
                    negmu = wp.tile([P, 1], f32, tag="negmu")
                    nc.vector.tensor_scalar(out=negmu[:], in0=rsum[:],
                                            scalar1=-1.0 / HC, scalar2=None,
                                            op0=Alu.mult)
                    cen = wp.tile([P, HC], bf, tag="cen")
                    nc.scalar.activation(cen[:], r[:], Act.Identity,
                                         bias=negmu[:])
                    vsum = wp.tile([P, 1], f32, tag="vsum")
                    junk = wp.tile([P, HC], bf, tag="junkA")
                    nc.vector.scalar_tensor_tensor(
                        out=junk[:], in0=cen[:], scalar=1.0, in1=cen[:],
                        op0=Alu.mult, op1=Alu.mult, accum_out=vsum[:])
                    sd = wp.tile([P, 1], f32, tag="sd")
                    nc.scalar.activation(sd[:], vsum[:], Act.Sqrt,
                                         bias=eps_sb[:], scale=1.0 / HC)
                    rstd = wp.tile([P, 1], f32, tag="rstd")
                    nc.vector.reciprocal(rstd[:], sd[:])
                    nc.scalar.activation(ego_sb[:, gt, :], cen[:], Act.Copy,
                                         scale=rstd[:])
                    egoT_ps = pstr.tile([P, HC], bf, tag="ptr")
                    nc.tensor.transpose(out=egoT_ps[:], in_=ego_sb[:, gt, :],
                                        identity=ident[:])
                    egoT_sb = wp.tile([HC, P], bf, tag="egoT")
                    nc.scalar.activation(egoT_sb[:], egoT_ps[:], Act.Copy)
                    hpst = ptgp.tile([P, GRP * HID], f32, tag="ptg")
                    nc.tensor.matmul(out=hpst[:, 0:HID], lhsT=egoT_sb[:],
                                     rhs=wlint_sb[:], start=True, stop=True)
                    nc.scalar.activation(hall_sb[:, gt, :], hpst[:, 0:HID],
                                         Act.Copy)
                # paired h shard -> DRAM (two contiguous-half DMAs)
                hr = hown[:].rearrange("(s p2) f -> p2 s f", p2=64)
                nc.sync.dma_start(hr[:, :, 0:HID], hall_sb[0:64, :, :])
                nc.sync.dma_start(hr[:, :, HID:P], hall_sb[64:P, :, :])
                if not _os.environ.get("KCCNOBAR"):
                    tc.strict_bb_all_engine_barrier()
                if not _os.environ.get("KSIM_NOCC"):
                    nc.gpsimd.collective_compute(
                        "AllGather",
                        mybir.AluOpType.bypass,
                        replica_groups=[list(range(NCORES))],
                        ins=[hown[:].opt()],
                        outs=[hgall[:].opt()],
                    )
                if not _os.environ.get("KCCNOBAR"):
                    tc.strict_bb_all_engine_barrier()

                # ================= Phase B =================
                chunks = {"E": {}, "O": {}}
                streams = {
                    "E": (colE_sb, SE, gpe, 0, 0),
                    "O": (colO_sb, SO, gpo, 1, HID),
                }

                def get_tile(stream, g):
                    """-> (chunk_tile, sub, p0): hc = chunk[:, sub, p0:p0+HID]"""
                    colsb, stot, pool, q, p0 = streams[stream]
                    c = g * P // CALL
                    sub = (g * P % CALL) // P
                    bufs = chunks[stream]
                    if c not in bufs:
                        n_i = min(CALL, stot - c * CALL)
                        n6 = n_i // P
                        hcb = pool.tile([P, CT, P], bf, tag="hc" + stream)
                        i0 = c * (CALL // 16)
                        i1 = i0 + (n_i + 15) // 16
                        if _os.environ.get("KNOGATHER"):
                            nc.sync.dma_start(
                                hcb[:, :n6, :],
                                hgall[0 : n6 * P, :].rearrange(
                                    "(t p) f -> p t f", p=P))
                        else:
                            nc.gpsimd.dma_gather(
                                hcb[:, :n6, :], hgall[:], colsb[:, i0:i1],
                                n_i, n_i, P, queue_num=q % NSWQ)
                        bufs[c] = hcb
                    return bufs[c], sub, p0

                schunks = {}

                def get_st05(gtile):
                    cs = gtile // SCH
                    if cs not in schunks:
                        n_t = min(SCH, NT - cs * SCH)
                        sb_ = spp.tile([P, SCH, P], f8, tag="sch")
                        if not _os.environ.get("KNOST05"):
                            nc.sync.dma_start(
                                sb_[:, :n_t, :],
                                sarr[:, cs * SCH * P : (cs * SCH + n_t) * P],
                            )
                        schunks[cs] = sb_
                    return schunks[cs][:, gtile % SCH, :]

                gcnt = {"E": 0, "O": 0}
                gt = 0
                for wi in range(NWIN):
                    ntile = T_E[wi] + T_O[wi]
                    if ntile == 0 or KSTAGE == 1:
                        nc.vector.memset(agg_sb[:, wi, :], 0.0)
                        continue
                    plan = []  # (stream, g, gtile)
                    g0 = {"E": gcnt["E"], "O": gcnt["O"]}
                    for stream, tcount in (("E", T_E[wi]), ("O", T_O[wi])):
                        for _ in range(tcount):
                            plan.append((stream, gcnt[stream], gt))
                            gcnt[stream] += 1
                            gt += 1
                    gt0 = gt - ntile

                    # on-chip S build, [e, d, t] layout (packed innermost
                    # on both operands -> DVE 2x mode eligible)
                    sall = sap.tile([P, P, MAXW], bf, tag="sall")
                    rdt_b = rdt_sb[:, gt0:gt0 + ntile].rearrange(
                        "p (o t) -> p o t", o=1).broadcast_to([P, P, ntile])
                    iot_b = iotar_sb[:].rearrange(
                        "p (d t) -> p d t", t=MAXW)[:, :, 0:ntile]
                    nc.vector.tensor_tensor(out=sall[:, :, 0:ntile],
                                            in0=rdt_b, in1=iot_b,
                                            op=Alu.is_equal)
                    if KSTAGE == 2:
                        for stream, tcount in (("E", T_E[wi]), ("O", T_O[wi])):
                            for j in range(tcount):
                                pass
                        for j in range(ntile):
                            get_tile(*plan[j][:2])
                            get_st05(gt0 + j)
                        nc.vector.memset(agg_sb[:, wi, :], 0.0)
                        for stream in ("E", "O"):
                            done = (g0[stream] * P) // CALL
                            for c in [c for c in chunks[stream] if c < done]:
                                del chunks[stream][c]
                        sdone = gt0 // SCH
                        for c in [c for c in schunks if c < sdone]:
                            del schunks[c]
                        continue

                    # tt in grouped PSUM; d per group via STT + reduce
                    ddwin = wp.tile([P, MAXW], bf, tag="ddwin")
                    ngrp = -(-ntile // GRP)
                    for gi in range(ngrp):
                        a = gi * GRP
                        g = min(GRP, ntile - a)
                        ptt = ptgp.tile([P, GRP * HID], f32, tag="ptg")
                        for j in range(g):
                            stream, gg, _ = plan[a + j]
                            hcb, sub, p0 = get_tile(stream, gg)
                            st05 = get_st05(gt0 + a + j)
                            nc.tensor.matmul(out=ptt[:, j * HID:(j + 1) * HID],
                                             lhsT=st05,
                                             rhs=hall_sb[:, wi, :],
                                             start=True, stop=False)
                            nc.tensor.matmul(out=ptt[:, j * HID:(j + 1) * HID],
                                             lhsT=ident[:],
                                             rhs=hcb[:, sub, p0:p0 + HID],
                                             start=False, stop=True)
                        rtt = wp.tile([P, GRP * HID], bf, tag="rtt")
                        wd_b = wd_sb[:].rearrange(
                            "p (o f) -> p o f", o=1).broadcast_to([P, g, HID])
                        nc.vector.scalar_tensor_tensor(
                            out=rtt[:].rearrange("p (t f) -> p t f", t=GRP)[:, 0:g, :],
                            in0=ptt[:].rearrange("p (t f) -> p t f", t=GRP)[:, 0:g, :],
                            scalar=0.0, in1=wd_b, op0=Alu.max, op1=Alu.mult)
                        with nc.allow_low_precision(reason="d in bf16"):
                            nc.vector.tensor_reduce(
                                out=ddwin[:, a:a + g],
                                in_=rtt[:].rearrange(
                                    "p (t f) -> p t f", t=GRP)[:, 0:g, :],
                                axis=mybir.AxisListType.X, op=Alu.add)

                    # att = sigmoid(d)
                    attw = wp.tile([P, MAXW], bf, tag="attw")
                    nc.scalar.activation(attw[:, 0:ntile], ddwin[:, 0:ntile],
                                         Act.Sigmoid)
                    if KSTAGE == 3:
                        nc.vector.memset(agg_sb[:, wi, :], 0.0)
                        for stream in ("E", "O"):
                            done = (g0[stream] * P) // CALL
                            for c in [c for c in chunks[stream] if c < done]:
                                del chunks[stream][c]
                        sdone = gt0 // SCH
                        for c in [c for c in schunks if c < sdone]:
                            del schunks[c]
                        continue

                    # xj batched per (stream, chunk) run
                    xjw = xjp.tile([P, MAXW, HID], bf, tag="xjw")
                    if _os.environ.get("KXJTILE"):
                        for ti in range(ntile):
                            stream, gg, _ = plan[ti]
                            hcb, sub, p0 = get_tile(stream, gg)
                            nc.vector.tensor_scalar(
                                out=xjw[:, ti, :],
                                in0=hcb[:, sub, p0:p0 + HID],
                                scalar1=attw[:, ti:ti + 1], scalar2=None,
                                op0=Alu.mult)
                    else:
                        ti = 0
                        while ti < ntile:
                            stream, gg, _ = plan[ti]
                            hcb, sub, p0 = get_tile(stream, gg)
                            L = 1
                            while (ti + L < ntile and plan[ti + L][0] == stream
                                   and plan[ti + L][1] == gg + L
                                   and sub + L < CT + 1 and (gg + L) * P // CALL
                                   == gg * P // CALL):
                                L += 1
                            att_b = attw[:, ti:ti + L].rearrange(
                                "p (t o) -> p t o", o=1).broadcast_to([P, L, HID])
                            nc.vector.tensor_tensor(
                                out=xjw[:, ti:ti + L, :],
                                in0=hcb[:, sub:sub + L, p0:p0 + HID],
                                in1=att_b, op=Alu.mult)
                            ti += L

                    # scatter (acc0/acc1 in separate PSUM banks)
                    acc0 = accp.tile([P, HID], f32, tag="acc0")
                    acc1 = accp.tile([P, HID], f32, tag="acc1")
                    for ti, (stream, gg, _) in enumerate(plan):
                        hcb, sub, p0 = get_tile(stream, gg)
                        st = ti == 0
                        sp = ti == ntile - 1
                        nc.tensor.matmul(out=acc0[:], lhsT=sall[:, :, ti],
                                         rhs=xjw[:, ti, :], start=st, stop=sp)
                        nc.tensor.matmul(out=acc1[:], lhsT=sall[:, :, ti],
                                         rhs=hcb[:, sub, p0:p0 + HID],
                                         start=st, stop=sp)
                    nc.scalar.activation(agg_sb[:, wi, 0:HID], acc0[:],
                                         Act.Copy)
                    with nc.allow_low_precision(reason="agg bf16"):
                        nc.vector.tensor_tensor(
                            out=agg_sb[:, wi, HID:HC], in0=acc1[:],
                            in1=agg_sb[:, wi, 0:HID], op=Alu.subtract)
                    # free consumed chunks
                    for stream in ("E", "O"):
                        done = (g0[stream] * P) // CALL
                        for c in [c for c in chunks[stream] if c < done]:
                            del chunks[stream][c]
                    sdone = gt0 // SCH
                    for c in [c for c in schunks if c < sdone]:
                        del schunks[c]

                # ================= Phase C =================
                # barrier keeps Phase C's Sqrt table swaps out of Phase B
                tc.strict_bb_all_engine_barrier()
                for wi in range(NWIN):
                    xh = wp.tile([P, HC], bf, tag="xh")
                    rsum = wp.tile([P, 1], f32, tag="rsum")
                    nc.scalar.activation(xh[:], agg_sb[:, wi, :], Act.Relu,
                                         accum_out=rsum[:])
                    negmu = wp.tile([P, 1], f32, tag="negmu")
                    nc.vector.tensor_scalar(out=negmu[:], in0=rsum[:],
                                            scalar1=-1.0 / HC, scalar2=None,
                                            op0=Alu.mult)
                    cen = wp.tile([P, HC], bf, tag="cen")
                    nc.scalar.activation(cen[:], xh[:], Act.Identity,
                                         bias=negmu[:])
                    vsum = wp.tile([P, 1], f32, tag="vsum")
                    junk = wp.tile([P, HC], bf, tag="junkA")
                    nc.vector.scalar_tensor_tensor(
                        out=junk[:], in0=cen[:], scalar=1.0, in1=cen[:],
                        op0=Alu.mult, op1=Alu.mult, accum_out=vsum[:])
                    sd = wp.tile([P, 1], f32, tag="sd")
                    nc.scalar.activation(sd[:], vsum[:], Act.Sqrt,
                                         bias=eps_sb[:], scale=1.0 / HC)
                    rstd = wp.tile([P, 1], f32, tag="rstd")
                    nc.vector.reciprocal(rstd[:], sd[:])
                    ln = wp.tile([P, HC], bf, tag="ln")
                    nc.scalar.activation(ln[:], cen[:], Act.Copy,
                                         scale=rstd[:])
                    xb = wp.tile([P, HC], bf, tag="xb")
                    with nc.allow_low_precision(reason="blend bf16"):
                        nc.vector.tensor_tensor(out=xb[:], in0=ln[:],
                                                in1=ego_sb[:, wi, :], op=Alu.add)
                    xbT_ps = pstr.tile([P, HC], bf, tag="ptr")
                    nc.tensor.transpose(out=xbT_ps[:], in_=xb[:], identity=ident[:])
                    xbT_sb = wp.tile([HC, P], bf, tag="xbT")
                    nc.scalar.activation(xbT_sb[:], xbT_ps[:], Act.Copy)
                    psOt = ptgp.tile([P, GRP * HID], f32, tag="ptg")
                    nc.tensor.matmul(out=psOt[:, 0:OUT], lhsT=xbT_sb[:],
                                     rhs=w2t_sb[:], start=True, stop=False)
                    nc.tensor.matmul(out=psOt[:, 0:OUT], lhsT=ones1[:],
                                     rhs=b2_sb[:], start=False, stop=True)
                    nc.vector.tensor_copy(o_sb[:, wi, :], psOt[:, 0:OUT])
                nc.sync.dma_start(
                    outd[:].rearrange("(t p) f -> p t f", p=P), o_sb[:]
                )
    nc.compile()
    return nc


def _get_compiled(key, T_E, T_O, reps):
    if key not in _cache:
        _cache[key] = _build(T_E, T_O, reps)
    return _cache[key]


def prepare(inputs, reps=1):
    """Host prep + build; returns (nc, in_maps, perms)."""
    g0 = np.asarray(inputs["g0"])
    beta0 = np.asarray(inputs["beta0"])
    g1 = np.asarray(inputs["g1"])
    beta1 = np.asarray(inputs["beta1"])
    assert np.allclose(g0, 1.0) and np.allclose(beta0, 0.0)
    assert np.allclose(g1, 1.0) and np.allclose(beta1, 0.0)
    in_maps, (T_E, T_O), perms = _host_prep(
        inputs["x"], inputs["edge_index"], inputs["W1"], inputs["b1"],
        inputs["Wlin"], inputs["Watt"], inputs["W2"], inputs["b2"],
    )
    key = (T_E, T_O, reps)
    nc = _get_compiled(key, list(T_E), list(T_O), reps)
    return nc, in_maps, perms


def kernel(**inputs) -> np.ndarray:
    from concourse.bass_utils import run_bass_kernel_spmd

    nc, in_maps, perms = prepare(inputs, reps=1)
    res = run_bass_kernel_spmd(nc, in_maps, list(range(NCORES)))
    full = np.empty((NP, OUT), np.float32)
    for k in range(NCORES):
        ok = res.results[k]["out"]          # [SH, OUT] slot-ordered
        for s in range(NWIN):
            w = perms[k][s]
            full[w * P : (w + 1) * P] = ok[s * P : (s + 1) * P]
    return full[:N]


# revision 37
# speedup vs baseline: 1.1670x; 1.1060x over previous
"""M2M-GNN (nn_M2MGNNPro) Trainium2 kernel, 8-core SPMD, bf16. v3.

Strategy (edge-parallel, destination-sharded, slot-permuted windows):
- Core k owns dest nodes [k*6272, (k+1)*6272) = 49 windows, processed in a
  per-core load-balancing slot order (windows sorted by edge count desc so
  the shared SPMD tile counts T[s] = max over cores are tight).
- Phase A (node-sharded): h0=relu(x@W1.T+b1), ego=LN(h0) (rstd via
  exp(-0.5*ln(var+eps)) so ScalarE stays on one act table), h=ego@Wlin.T
  per slot; h shard packed PAIRED into hown [SH/2, 128] rows
  [h_p | h_{p+64}] (two contiguous-partition DMAs), AllGather ->
  hgall [NP/2, 128] (6.4MB vs 12.8MB unpaired).
- Phase B per window: edges split by col parity ((col%128)//64) into E/O
  streams (gather idx = paired row < 25088, int16-safe, no 32768 split);
  gpsimd dma_gather streams fetch h_col rows in CALL-row chunks (parity
  selects col half). Per tile: gather-mm (streamed ST05 one-hot lhsT) +
  ident-mm build tt in grouped PSUM [P, G*64]; one batched DVE
  STT(relu*wd) + grouped tensor_reduce -> d; att = 1/(1+exp(-d)) (Exp on
  ScalarE, add+reciprocal on DVE); xj = att*hc batched per chunk-run via
  broadcast AP; scatter via PE matmuls with S built ON-CHIP (one batched
  DVE is_equal per window vs iota). agg half1 = sum(hc) - half0.
- Phase C: relu(agg), LN (ln/exp), blend with ego (0.5 into W2), GEMM W2;
  outputs staged in SBUF, one DMA; host unpermutes window slots.
All ScalarE funcs in {Relu, Copy, Ln, Exp} -> single act table load.
"""
import os as _os_mod

import numpy as np

N = 50000
E = 800000
IN = 128
HID = 64
C = 2
HC = 128
OUT = 40
BETA = 0.5
TEMP = 1.0
EPS = 1e-5

NCORES = 8
P = 128
NP = 50176            # 392 tiles of 128
SH = NP // NCORES     # 6272 nodes/core, 49 windows
NWIN = SH // P        # 49
SHROW = SH // 2       # paired rows per core (3136)

CALL = int(_os_mod.environ.get("KCALL", "1024"))  # gather rows per call
CT = CALL // P
SCH = 8               # st05 tiles per streamed chunk
GRP = 8               # tt tiles per PSUM group

_cache = {}


def _bf16():
    import ml_dtypes

    return ml_dtypes.bfloat16


def _host_prep(x, edge_index, W1, b1, Wlin, Watt, W2, b2):
    bf16 = _bf16()
    x = np.asarray(x, np.float32)
    row = np.asarray(edge_index[0], np.int64)
    col = np.asarray(edge_index[1], np.int64)

    x_pad = np.zeros((NP, IN), np.float32)
    x_pad[:N] = x

    wglob = row // P                      # dest window 0..391
    wcount = np.bincount(wglob, minlength=NP // P)
    # per-core slot permutation: windows by edge count desc
    perms = []                            # perms[k][s] = global window
    inv_slot = np.zeros(NP // P, np.int64)
    for k in range(NCORES):
        wins = np.arange(k * NWIN, (k + 1) * NWIN)
        order = wins[np.argsort(-wcount[wins], kind="stable")]
        perms.append(order)
        inv_slot[order] = np.arange(NWIN)

    # gather row + parity for cols (remapped, paired (p, p+64))
    p_c = col % P
    w_c = col // P
    k_c = w_c // NWIN
    grow = (k_c * SHROW + inv_slot[w_c] * 64 + (p_c % 64)).astype(np.int64)
    par = (p_c // 64).astype(np.int64)
    rd = (row % P).astype(np.int64)
    core = wglob // NWIN
    slot = inv_slot[wglob]

    # bucket edges per (core, slot, parity), sorted by rd
    buckets = {}
    for k in range(NCORES):
        mk = core == k
        sk, pk, gk, rk = slot[mk], par[mk], grow[mk], rd[mk]
        for s in range(NWIN):
            ms = sk == s
            for pa in range(2):
                m = ms & (pk == pa)
                g, r = gk[m], rk[m]
                o = np.argsort(r, kind="stable")
                buckets[(k, s, pa)] = (g[o], r[o])

    T_E = np.zeros(NWIN, np.int64)
    T_O = np.zeros(NWIN, np.int64)
    for s in range(NWIN):
        T_E[s] = max(-(-len(buckets[(k, s, 0)][0]) // P) for k in range(NCORES))
        T_O[s] = max(-(-len(buckets[(k, s, 1)][0]) // P) for k in range(NCORES))
    MAXW = int(max(T_E[s] + T_O[s] for s in range(NWIN)))
    SE = int(T_E.sum()) * P
    SO = int(T_O.sum()) * P
    NT = int(T_E.sum() + T_O.sum())

    def wrap16(a):
        n = len(a)
        pad = (-n) % 16
        a = np.concatenate([a, np.zeros(pad, np.int16)])
        return np.tile(a.reshape(-1, 16).T, (8, 1))

    ar = np.arange(P)
    in_maps = []
    for k in range(NCORES):
        colE = np.zeros(SE, np.int16)
        colO = np.zeros(SO, np.int16)
        rd_all = np.full(NT * P, 200.0, np.float32)
        oe = oo = 0
        gt = 0
        for s in range(NWIN):
            (ge, re) = buckets[(k, s, 0)]
            (go, ro) = buckets[(k, s, 1)]
            ne, no = len(ge), len(go)
            colE[oe : oe + ne] = ge.astype(np.int16)
            colO[oo : oo + no] = go.astype(np.int16)
            rd_all[gt * P : gt * P + ne] = re.astype(np.float32)
            gt += int(T_E[s])
            rd_all[gt * P : gt * P + no] = ro.astype(np.float32)
            gt += int(T_O[s])
            oe += int(T_E[s]) * P
            oo += int(T_O[s]) * P

        # host-built ST05 [d, e] = 0.5*(d == rd[e]) per tile -> [P, NT*P]
        import ml_dtypes as _mld
        f8 = _mld.float8_e4m3
        rdm = rd_all.reshape(NT, P)
        st05 = ((ar[:, None, None] == rdm[None, :, :]).astype(f8)
                * f8(0.5)).reshape(P, NT * P)

        xk = np.empty((SH, IN), np.float32)
        for s in range(NWIN):
            w = perms[k][s]
            xk[s * P : (s + 1) * P] = x_pad[w * P : (w + 1) * P]
        in_maps.append(
            {
                "xT": np.ascontiguousarray(xk.T.astype(bf16)),
                "colE": wrap16(colE),
                "colO": wrap16(colO),
                "rdt": np.ascontiguousarray(
                    rd_all.reshape(NT, P).T.astype(bf16)
                ),
                "sarr": np.ascontiguousarray(st05),
            }
        )
    wd = (np.asarray(Watt[0]) - np.asarray(Watt[1])).astype(np.float32)
    shared = {
        "w1t": np.asarray(W1, np.float32).T.astype(bf16).copy(),
        "b1row": np.asarray(b1, np.float32)[None, :].astype(bf16),
        "wlint": np.asarray(Wlin, np.float32).T.astype(bf16).copy(),
        "wdrep": np.tile(wd[None, :], (P, 1)).astype(bf16),
        "iotac": np.tile(
            np.arange(P, dtype=np.float32)[None, :], (P, 1)
        ).astype(bf16),
        "iotar": np.tile(
            np.repeat(np.arange(P, dtype=np.float32), MAXW)[None, :], (P, 1)
        ).astype(bf16),
        "w2t": ((1.0 - BETA) * np.asarray(W2, np.float32).T).astype(bf16).copy(),
        "b2row": np.asarray(b2, np.float32)[None, :].astype(bf16),
    }
    for im in in_maps:
        im.update(shared)
    return in_maps, (tuple(T_E.tolist()), tuple(T_O.tolist())), perms


def _build(T_E, T_O, reps=1):
    import concourse.bacc as bacc
    import concourse.mybir as mybir
    import concourse.tile as tile
    from concourse.library_config import mlp
    from concourse.masks import make_identity

    f32 = mybir.dt.float32
    bf = mybir.dt.bfloat16
    i16 = mybir.dt.int16
    Alu = mybir.AluOpType
    Act = mybir.ActivationFunctionType

    SE = sum(T_E) * P
    SO = sum(T_O) * P
    NT = sum(T_E) + sum(T_O)
    MAXW = max(te + to for te, to in zip(T_E, T_O))

    NSWQ = int(_os_mod.environ.get("KSWQ", "2"))
    KSCR = int(_os_mod.environ.get("KSCR", "16384"))
    KSTAGE = int(_os_mod.environ.get("KSTAGE", "0"))  # 0=full bisect gates
    nc = bacc.Bacc("TRN2", num_devices=NCORES, num_swdge_queues=NSWQ,
                   dynamic_dma_scratch_size=KSCR)
    xT = nc.dram_tensor("xT", [IN, SH], bf, kind="ExternalInput")
    colE = nc.dram_tensor("colE", [P, (SE + 15) // 16], i16, kind="ExternalInput")
    colO = nc.dram_tensor("colO", [P, (SO + 15) // 16], i16, kind="ExternalInput")
    rdt = nc.dram_tensor("rdt", [P, NT], bf, kind="ExternalInput")
    f8 = mybir.dt.float8e4
    sarr = nc.dram_tensor("sarr", [P, NT * P], f8, kind="ExternalInput")
    w1t = nc.dram_tensor("w1t", [IN, HC], bf, kind="ExternalInput")
    b1row = nc.dram_tensor("b1row", [1, HC], bf, kind="ExternalInput")
    wlint = nc.dram_tensor("wlint", [HC, HID], bf, kind="ExternalInput")
    wdrep = nc.dram_tensor("wdrep", [P, HID], bf, kind="ExternalInput")
    iotac = nc.dram_tensor("iotac", [P, P], bf, kind="ExternalInput")
    iotar = nc.dram_tensor("iotar", [P, P * MAXW], bf, kind="ExternalInput")
    w2t = nc.dram_tensor("w2t", [HC, OUT], bf, kind="ExternalInput")
    b2row = nc.dram_tensor("b2row", [1, OUT], bf, kind="ExternalInput")
    hown = nc.dram_tensor("hown", [SHROW, P], bf)   # paired local shard
    hgall = nc.dram_tensor("hgall", [NP // 2, P], bf)  # allgathered table
    outd = nc.dram_tensor("out", [SH, OUT], f32, kind="ExternalOutput")
    import os as _os

    with tile.TileContext(nc) as tc:
        with (
            tc.tile_pool(name="const", bufs=1) as cp,
            tc.tile_pool(name="work", bufs=6) as wp,
            tc.tile_pool(name="gE", bufs=4) as gpe,
            tc.tile_pool(name="gO", bufs=4) as gpo,
            tc.tile_pool(name="sp", bufs=6) as spp,
            tc.tile_pool(name="sall", bufs=2) as sap,
            tc.tile_pool(name="xjp", bufs=2) as xjp,
            tc.tile_pool(name="pstr", bufs=1, space="PSUM") as pstr,
            tc.tile_pool(name="ptg", bufs=2, space="PSUM") as ptgp,
            tc.tile_pool(name="acc", bufs=2, space="PSUM") as accp,
        ):
            nc.gpsimd.load_library(mlp)
            # ---- constants to SBUF ----
            w1t_sb = cp.tile([IN, HC], bf, tag="w1t")
            b1_sb = cp.tile([1, HC], bf, tag="b1")
            wlint_sb = cp.tile([HC, HID], bf, tag="wlt")
            wd_sb = cp.tile([P, HID], bf, tag="wd")
            iota_sb = cp.tile([P, P], bf, tag="iota")
            iotar_sb = cp.tile([P, P * MAXW], bf, tag="iotar")
            w2t_sb = cp.tile([HC, OUT], bf, tag="w2t")
            b2_sb = cp.tile([1, OUT], bf, tag="b2")
            colE_sb = cp.tile([P, (SE + 15) // 16], i16, tag="colE")
            colO_sb = cp.tile([P, (SO + 15) // 16], i16, tag="colO")
            rdt_sb = cp.tile([P, NT], bf, tag="rdt")
            xT_sb = cp.tile([IN, SH], bf, tag="xT")
            for sb, dr in (
                (w1t_sb, w1t), (b1_sb, b1row), (wlint_sb, wlint),
                (wd_sb, wdrep), (iota_sb, iotac), (iotar_sb, iotar),
                (w2t_sb, w2t),
                (b2_sb, b2row), (colE_sb, colE), (colO_sb, colO),
                (rdt_sb, rdt), (xT_sb, xT),
            ):
                nc.sync.dma_start(sb[:], dr[:])
            ident = cp.tile([P, P], bf, tag="ident")
            make_identity(nc, ident[:])
            ones1 = cp.tile([1, P], bf, tag="ones1")
            nc.vector.memset(ones1[:], 1.0)
            eps_sb = cp.tile([P, 1], f32, tag="eps")
            nc.vector.memset(eps_sb[:], EPS)
            ego_sb = cp.tile([P, NWIN, HC], bf, tag="ego")
            agg_sb = cp.tile([P, NWIN, HC], bf, tag="agg")
            hall_sb = cp.tile([P, NWIN, HID], bf, tag="hall")
            o_sb = cp.tile([P, NWIN, OUT], f32, tag="osb")

            for rep in range(reps):
                tc.strict_bb_all_engine_barrier()
                # ================= Phase A =================
                for gt in range(NWIN):
                    psAt = ptgp.tile([P, GRP * HID], f32, tag="ptg")
                    psA = psAt[:, 0:HC]
                    nc.tensor.matmul(out=psA, lhsT=xT_sb[:, gt * P : (gt + 1) * P],
                                     rhs=w1t_sb[:], start=True, stop=False)
                    nc.tensor.matmul(out=psA, lhsT=ones1[:], rhs=b1_sb[:],
                                     start=False, stop=True)
                    r = wp.tile([P, HC], bf, tag="r")
                    rsum = wp.tile([P, 1], f32, tag="rsum")
                    nc.scalar.activation(r[:], psA, Act.Relu, accum_out=rsum[:])
                    negmu = wp.tile([P, 1], f32, tag="negmu")
                    nc.vector.tensor_scalar(out=negmu[:], in0=rsum[:],
                                            scalar1=-1.0 / HC, scalar2=None,
                                            op0=Alu.mult)
                    cen = wp.tile([P, HC], bf, tag="cen")
                    nc.scalar.activation(cen[:], r[:], Act.Identity,
                                         bias=negmu[:])
                    vsum = wp.tile([P, 1], f32, tag="vsum")
                    junk = wp.tile([P, HC], bf, tag="junkA")
                    nc.vector.scalar_tensor_tensor(
                        out=junk[:], in0=cen[:], scalar=1.0, in1=cen[:],
                        op0=Alu.mult, op1=Alu.mult, accum_out=vsum[:])
                    sd = wp.tile([P, 1], f32, tag="sd")
                    nc.scalar.activation(sd[:], vsum[:], Act.Sqrt,
                                         bias=eps_sb[:], scale=1.0 / HC)
                    rstd = wp.tile([P, 1], f32, tag="rstd")
                    nc.vector.reciprocal(rstd[:], sd[:])
                    nc.scalar.activation(ego_sb[:, gt, :], cen[:], Act.Copy,
                                         scale=rstd[:])
                    egoT_ps = pstr.tile([P, HC], bf, tag="ptr")
                    nc.tensor.transpose(out=egoT_ps[:], in_=ego_sb[:, gt, :],
                                        identity=ident[:])
                    egoT_sb = wp.tile([HC, P], bf, tag="egoT")
                    nc.scalar.activation(egoT_sb[:], egoT_ps[:], Act.Copy)
                    hpst = ptgp.tile([P, GRP * HID], f32, tag="ptg")
                    nc.tensor.matmul(out=hpst[:, 0:HID], lhsT=egoT_sb[:],
                                     rhs=wlint_sb[:], start=True, stop=True)
                    nc.scalar.activation(hall_sb[:, gt, :], hpst[:, 0:HID],
                                         Act.Copy)
                # paired h shard -> DRAM (two contiguous-half DMAs)
                hr = hown[:].rearrange("(s p2) f -> p2 s f", p2=64)
                nc.sync.dma_start(hr[:, :, 0:HID], hall_sb[0:64, :, :])
                nc.sync.dma_start(hr[:, :, HID:P], hall_sb[64:P, :, :])
                if not _os.environ.get("KCCNOBAR"):
                    tc.strict_bb_all_engine_barrier()
                if not _os.environ.get("KSIM_NOCC"):
                    nc.gpsimd.collective_compute(
                        "AllGather",
                        mybir.AluOpType.bypass,
                        replica_groups=[list(range(NCORES))],
                        ins=[hown[:].opt()],
                        outs=[hgall[:].opt()],
                    )
                if not _os.environ.get("KCCNOBAR"):
                    tc.strict_bb_all_engine_barrier()

                # ================= Phase B =================
                chunks = {"E": {}, "O": {}}
                streams = {
                    "E": (colE_sb, SE, gpe, 0, 0),
                    "O": (colO_sb, SO, gpo, 1, HID),
                }

                def get_tile(stream, g):
                    """-> (chunk_tile, sub, p0): hc = chunk[:, sub, p0:p0+HID]"""
                    colsb, stot, pool, q, p0 = streams[stream]
                    c = g * P // CALL
                    sub = (g * P % CALL) // P
                    bufs = chunks[stream]
                    if c not in bufs:
                        n_i = min(CALL, stot - c * CALL)
                        n6 = n_i // P
                        hcb = pool.tile([P, CT, P], bf, tag="hc" + stream)
                        i0 = c * (CALL // 16)
                        i1 = i0 + (n_i + 15) // 16
                        if _os.environ.get("KNOGATHER"):
                            nc.sync.dma_start(
                                hcb[:, :n6, :],
                                hgall[0 : n6 * P, :].rearrange(
                                    "(t p) f -> p t f", p=P))
                        else:
                            nc.gpsimd.dma_gather(
                                hcb[:, :n6, :], hgall[:], colsb[:, i0:i1],
                                n_i, n_i, P, queue_num=q % NSWQ)
                        bufs[c] = hcb
                    return bufs[c], sub, p0

                schunks = {}

                def get_st05(gtile):
                    cs = gtile // SCH
                    if cs not in schunks:
                        n_t = min(SCH, NT - cs * SCH)
                        sb_ = spp.tile([P, SCH, P], f8, tag="sch")
                        if not _os.environ.get("KNOST05"):
                            nc.sync.dma_start(
                                sb_[:, :n_t, :],
                                sarr[:, cs * SCH * P : (cs * SCH + n_t) * P],
                            )
                        schunks[cs] = sb_
                    return schunks[cs][:, gtile % SCH, :]

                gcnt = {"E": 0, "O": 0}
                gt = 0
                for wi in range(NWIN):
                    ntile = T_E[wi] + T_O[wi]
                    if ntile == 0 or KSTAGE == 1:
                        nc.vector.memset(agg_sb[:, wi, :], 0.0)
                        continue
                    plan = []  # (stream, g, gtile)
                    g0 = {"E": gcnt["E"], "O": gcnt["O"]}
                    for stream, tcount in (("E", T_E[wi]), ("O", T_O[wi])):
                        for _ in range(tcount):
                            plan.append((stream, gcnt[stream], gt))
                            gcnt[stream] += 1
                            gt += 1
                    gt0 = gt - ntile

                    # on-chip S build, [e, d, t] layout (packed innermost
                    # on both operands -> DVE 2x mode eligible)
                    sall = sap.tile([P, P, MAXW], bf, tag="sall")
                    rdt_b = rdt_sb[:, gt0:gt0 + ntile].rearrange(
                        "p (o t) -> p o t", o=1).broadcast_to([P, P, ntile])
                    iot_b = iotar_sb[:].rearrange(
                        "p (d t) -> p d t", t=MAXW)[:, :, 0:ntile]
                    nc.vector.tensor_tensor(out=sall[:, :, 0:ntile],
                                            in0=rdt_b, in1=iot_b,
                                            op=Alu.is_equal)
                    if KSTAGE == 2:
                        for stream, tcount in (("E", T_E[wi]), ("O", T_O[wi])):
                            for j in range(tcount):
                                pass
                        for j in range(ntile):
                            get_tile(*plan[j][:2])
                            get_st05(gt0 + j)
                        nc.vector.memset(agg_sb[:, wi, :], 0.0)
                        for stream in ("E", "O"):
                            done = (g0[stream] * P) // CALL
                            for c in [c for c in chunks[stream] if c < done]:
                                del chunks[stream][c]
                        sdone = gt0 // SCH
                        for c in [c for c in schunks if c < sdone]:
                            del schunks[c]
                        continue

                    # tt in grouped PSUM; d per group via STT + reduce
                    ddwin = wp.tile([P, MAXW], f32, tag="ddwin")
                    ngrp = -(-ntile // GRP)
                    for gi in range(ngrp):
                        a = gi * GRP
                        g = min(GRP, ntile - a)
                        ptt = ptgp.tile([P, GRP * HID], f32, tag="ptg")
                        for j in range(g):
                            stream, gg, _ = plan[a + j]
                            hcb, sub, p0 = get_tile(stream, gg)
                            st05 = get_st05(gt0 + a + j)
                            nc.tensor.matmul(out=ptt[:, j * HID:(j + 1) * HID],
                                             lhsT=st05,
                                             rhs=hall_sb[:, wi, :],
                                             start=True, stop=False)
                            nc.tensor.matmul(out=ptt[:, j * HID:(j + 1) * HID],
                                             lhsT=ident[:],
                                             rhs=hcb[:, sub, p0:p0 + HID],
                                             start=False, stop=True)
                        rtt = wp.tile([P, GRP * HID], bf, tag="rtt")
                        wd_b = wd_sb[:].rearrange(
                            "p (o f) -> p o f", o=1).broadcast_to([P, g, HID])
                        nc.vector.scalar_tensor_tensor(
                            out=rtt[:].rearrange("p (t f) -> p t f", t=GRP)[:, 0:g, :],
                            in0=ptt[:].rearrange("p (t f) -> p t f", t=GRP)[:, 0:g, :],
                            scalar=0.0, in1=wd_b, op0=Alu.max, op1=Alu.mult)
                        nc.vector.tensor_reduce(
                            out=ddwin[:, a:a + g],
                            in_=rtt[:].rearrange(
                                "p (t f) -> p t f", t=GRP)[:, 0:g, :],
                            axis=mybir.AxisListType.X, op=Alu.add)

                    # att = sigmoid(d)
                    attw = wp.tile([P, MAXW], bf, tag="attw")
                    nc.scalar.activation(attw[:, 0:ntile], ddwin[:, 0:ntile],
                                         Act.Sigmoid)
                    if KSTAGE == 3:
                        nc.vector.memset(agg_sb[:, wi, :], 0.0)
                        for stream in ("E", "O"):
                            done = (g0[stream] * P) // CALL
                            for c in [c for c in chunks[stream] if c < done]:
                                del chunks[stream][c]
                        sdone = gt0 // SCH
                        for c in [c for c in schunks if c < sdone]:
                            del schunks[c]
                        continue

                    # xj batched per (stream, chunk) run
                    xjw = xjp.tile([P, MAXW, HID], bf, tag="xjw")
                    if _os.environ.get("KXJTILE"):
                        for ti in range(ntile):
                            stream, gg, _ = plan[ti]
                            hcb, sub, p0 = get_tile(stream, gg)
                            nc.vector.tensor_scalar(
                                out=xjw[:, ti, :],
                                in0=hcb[:, sub, p0:p0 + HID],
                                scalar1=attw[:, ti:ti + 1], scalar2=None,
                                op0=Alu.mult)
                    else:
                        ti = 0
                        while ti < ntile:
                            stream, gg, _ = plan[ti]
                            hcb, sub, p0 = get_tile(stream, gg)
                            L = 1
                            while (ti + L < ntile and plan[ti + L][0] == stream
                                   and plan[ti + L][1] == gg + L
                                   and sub + L < CT + 1 and (gg + L) * P // CALL
                                   == gg * P // CALL):
                                L += 1
                            att_b = attw[:, ti:ti + L].rearrange(
                                "p (t o) -> p t o", o=1).broadcast_to([P, L, HID])
                            nc.vector.tensor_tensor(
                                out=xjw[:, ti:ti + L, :],
                                in0=hcb[:, sub:sub + L, p0:p0 + HID],
                                in1=att_b, op=Alu.mult)
                            ti += L

                    # scatter (acc0/acc1 in separate PSUM banks)
                    acc0 = accp.tile([P, HID], f32, tag="acc0")
                    acc1 = accp.tile([P, HID], f32, tag="acc1")
                    for ti, (stream, gg, _) in enumerate(plan):
                        hcb, sub, p0 = get_tile(stream, gg)
                        st = ti == 0
                        sp = ti == ntile - 1
                        nc.tensor.matmul(out=acc0[:], lhsT=sall[:, :, ti],
                                         rhs=xjw[:, ti, :], start=st, stop=sp)
                        nc.tensor.matmul(out=acc1[:], lhsT=sall[:, :, ti],
                                         rhs=hcb[:, sub, p0:p0 + HID],
                                         start=st, stop=sp)
                    nc.scalar.activation(agg_sb[:, wi, 0:HID], acc0[:],
                                         Act.Copy)
                    with nc.allow_low_precision(reason="agg bf16"):
                        nc.vector.tensor_tensor(
                            out=agg_sb[:, wi, HID:HC], in0=acc1[:],
                            in1=agg_sb[:, wi, 0:HID], op=Alu.subtract)
                    # free consumed chunks
                    for stream in ("E", "O"):
                        done = (g0[stream] * P) // CALL
                        for c in [c for c in chunks[stream] if c < done]:
                            del chunks[stream][c]
                    sdone = gt0 // SCH
                    for c in [c for c in schunks if c < sdone]:
                        del schunks[c]

                # ================= Phase C =================
                # barrier keeps Phase C's Sqrt table swaps out of Phase B
                tc.strict_bb_all_engine_barrier()
                for wi in range(NWIN):
                    xh = wp.tile([P, HC], bf, tag="xh")
                    rsum = wp.tile([P, 1], f32, tag="rsum")
                    nc.scalar.activation(xh[:], agg_sb[:, wi, :], Act.Relu,
                                         accum_out=rsum[:])
                    negmu = wp.tile([P, 1], f32, tag="negmu")
                    nc.vector.tensor_scalar(out=negmu[:], in0=rsum[:],
                                            scalar1=-1.0 / HC, scalar2=None,
                                            op0=Alu.mult)
                    cen = wp.tile([P, HC], bf, tag="cen")
                    nc.scalar.activation(cen[:], xh[:], Act.Identity,
                                         bias=negmu[:])
                    vsum = wp.tile([P, 1], f32, tag="vsum")
                    junk = wp.tile([P, HC], bf, tag="junkA")
                    nc.vector.scalar_tensor_tensor(
                        out=junk[:], in0=cen[:], scalar=1.0, in1=cen[:],
                        op0=Alu.mult, op1=Alu.mult, accum_out=vsum[:])
                    sd = wp.tile([P, 1], f32, tag="sd")
                    nc.scalar.activation(sd[:], vsum[:], Act.Sqrt,
                                         bias=eps_sb[:], scale=1.0 / HC)
                    rstd = wp.tile([P, 1], f32, tag="rstd")
                    nc.vector.reciprocal(rstd[:], sd[:])
                    ln = wp.tile([P, HC], bf, tag="ln")
                    nc.scalar.activation(ln[:], cen[:], Act.Copy,
                                         scale=rstd[:])
                    xb = wp.tile([P, HC], bf, tag="xb")
                    with nc.allow_low_precision(reason="blend bf16"):
                        nc.vector.tensor_tensor(out=xb[:], in0=ln[:],
                                                in1=ego_sb[:, wi, :], op=Alu.add)
                    xbT_ps = pstr.tile([P, HC], bf, tag="ptr")
                    nc.tensor.transpose(out=xbT_ps[:], in_=xb[:], identity=ident[:])
                    xbT_sb = wp.tile([HC, P], bf, tag="xbT")
                    nc.scalar.activation(xbT_sb[:], xbT_ps[:], Act.Copy)
                    psOt = ptgp.tile([P, GRP * HID], f32, tag="ptg")
                    nc.tensor.matmul(out=psOt[:, 0:OUT], lhsT=xbT_sb[:],
                                     rhs=w2t_sb[:], start=True, stop=False)
                    nc.tensor.matmul(out=psOt[:, 0:OUT], lhsT=ones1[:],
                                     rhs=b2_sb[:], start=False, stop=True)
                    nc.vector.tensor_copy(o_sb[:, wi, :], psOt[:, 0:OUT])
                nc.sync.dma_start(
                    outd[:].rearrange("(t p) f -> p t f", p=P), o_sb[:]
                )
    nc.compile()
    return nc


def _get_compiled(key, T_E, T_O, reps):
    if key not in _cache:
        _cache[key] = _build(T_E, T_O, reps)
    return _cache[key]


def prepare(inputs, reps=1):
    """Host prep + build; returns (nc, in_maps, perms)."""
    g0 = np.asarray(inputs["g0"])
    beta0 = np.asarray(inputs["beta0"])
    g1 = np.asarray(inputs["g1"])
    beta1 = np.asarray(inputs["beta1"])
    assert np.allclose(g0, 1.0) and np.allclose(beta0, 0.0)
    assert np.allclose(g1, 1.0) and np.allclose(beta1, 0.0)
    in_maps, (T_E, T_O), perms = _host_prep(
        inputs["x"], inputs["edge_index"], inputs["W1"], inputs["b1"],
        inputs["Wlin"], inputs["Watt"], inputs["W2"], inputs["b2"],
    )
    key = (T_E, T_O, reps)
    nc = _get_compiled(key, list(T_E), list(T_O), reps)
    return nc, in_maps, perms


def kernel(**inputs) -> np.ndarray:
    from concourse.bass_utils import run_bass_kernel_spmd

    nc, in_maps, perms = prepare(inputs, reps=1)
    res = run_bass_kernel_spmd(nc, in_maps, list(range(NCORES)))
    full = np.empty((NP, OUT), np.float32)
    for k in range(NCORES):
        ok = res.results[k]["out"]          # [SH, OUT] slot-ordered
        for s in range(NWIN):
            w = perms[k][s]
            full[w * P : (w + 1) * P] = ok[s * P : (s + 1) * P]
    return full[:N]
